# revision 3
# baseline (speedup 1.0000x reference)
"""Sparse 3D conv (gather -> matmul -> relu) via GPSIMD ap_gather, 8 cores.

out[n] = relu(sum_k feats[kmap[k,n]] @ W[k]), sentinel index N contributes 0.

Plan (data-parallel over voxels, no collectives):
  HOST:
    - Reconstruct a raster (z-order) voxel ordering from kmap alone: BFS over
      the 26-neighbor graph propagates exact (x,y,z) offsets, so each
      connected component gets consistent coords; sort by (component, lin).
      Neighbor rank deltas are then bounded by ~3300 (measured 3292).
    - Each core owns NPC consecutive sorted voxels. Its param slab is the
      transposed feature matrix FT[32, NPC + 2*H2] (halo'd, zero-padded).
    - Per supertile of 1024 voxels: int16 window-relative gather indices for
      the 27 taps, split into 4 quarters of 7 tap-blocks (Q3 has 6 + pad).
      Invalid taps -> index 0 (a permanently-zero window column).
  DEVICE (per supertile s):
    - Rolling circular window W[128, 1+CIRC] f32 = FT columns replicated on
      4x32 partitions; col(r) = 1 + r % CIRC; fixed schedule: update s writes
      rows (s*1024+3456 .. (s+1)*1024+3456], identical ranges on all cores.
      Two alternating windows (A/B) so updates overlap gathers.
    - ONE gpsimd.ap_gather (channels=128, d=1, num_idxs=7168): each 16-lane
      Q7 core gathers its quarter's tap stream; H[128, 7*1024] lands
      matmul-ready (partition = quarter-channel, col = block*1024 + voxel).
    - 14 matmuls K=128x512 (bf16): stationary = 4 stacked tap
      weights [128, 64] (zeros for missing), rhs = H block, PSUM accumulate.
    - ACT relu PSUM -> SBUF, DMA out as outT[64, positions].
  HOST: transpose, un-permute, drop pad rows.
"""

import numpy as np

import concourse.bass as bass
import concourse.mybir as mybir
import concourse.tile as tile
from concourse import bacc, library_config
from concourse.bass_utils import run_bass_kernel_spmd

# --- tail-drain wait splitting (same workaround as baseline kernel) --------


def _split_drain_and_barrier(self, tick_clock, wait_clock):
    nc = self.nc
    collector = nc.sync.nop(nofuse=True)
    wait_clock.add_sem_waits(
        collector.ins, tile.ScopedClock({None: tick_clock.global_clock})
    )
    si = collector.ins.sync_info
    waits = list(si.on_wait) if si is not None and si.on_wait else []
    if len(waits) > 1:
        collector.ins.sync_info = mybir.SyncInfo(
            on_wait=waits[:1], on_update=list(si.on_update or [])
        )
        for w in waits[1:]:
            extra = nc.sync.nop(nofuse=True)
            extra.ins.sync_info = mybir.SyncInfo(on_wait=[w], on_update=[])
    nc.sync.drain()
    nc.all_engine_barrier()
    popped = nc._tile_sem_poison_stack.pop()
    assert popped is self._sem_poison
    nc.clear_and_free_semaphores(list(self.sems.allocated().values()))
    nc.all_engine_barrier()


tile.TileContext._drain_and_barrier = _split_drain_and_barrier

# --- problem constants ----------------------------------------------------
N = 400000
INC = 32
OUTC = 64
K3 = 27
NCORES = 8
P = 128

SUPER = 1024
NSUP = 49
NPC = NSUP * SUPER          # 50176 voxels per core
NTOT = NCORES * NPC         # 401408 padded voxel count

NBLK = 7                    # tap blocks per quarter (Q3: 6 real + 1 pad)
NIDX = NBLK * SUPER         # 7168 gather indices per Q7 core per supertile
QTAPS = [list(range(0, 7)), list(range(7, 14)),
         list(range(14, 21)), list(range(21, 27))]

BAND = 3456                 # virtual-row reach above a supertile
CIRC = 6912                 # circular window length (cols 1..CIRC)
WINQ = CIRC + 1             # + permanently-zero col 0
H2 = 2560                   # slab col offset: slab col = virtual row + H2
SLAB = NSUP * SUPER + BAND + H2 + 64   # virtual-row slab columns
HFT = 6144                  # FT halo for per-quarter shifted slab builds

F32 = mybir.dt.float32
F32R = mybir.dt.float32r
I16 = mybir.dt.int16


def _cover_hi(s):
    return (s + 1) * SUPER + BAND  # highest row covered after update s


def _win_slices(r0, r1):
    """Rows [r0, r1) -> list of (win_col_start, slab_rel_start, length)."""
    out = []
    r = r0
    while r < r1:
        c = 1 + (r % CIRC)
        ln = min(r1 - r, CIRC + 1 - c)
        out.append((c, r, ln))
        r += ln
    return out


def build_nc(treps=1, no_mm=False, win_elems=WINQ, bf16_mm=True):
    nc = bacc.Bacc("TRN2", target_bir_lowering=False, debug=False)
    fp = nc.declare_dram_parameter("fp", [P, SLAB], F32, isOutput=False)
    idx = nc.declare_dram_parameter("idx", [NSUP, P, NIDX // 16], I16, isOutput=False)
    wstk = nc.declare_dram_parameter("wstk", [P, NBLK * OUTC], F32, isOutput=False)
    outT = nc.declare_dram_parameter("outT", [OUTC, NPC], F32, isOutput=True)

    def upd_window(win, r0, r1):
        """DMA slab rows [r0, r1) into circular window cols (slab is already
        replicated x4 across the 128 partitions by the host)."""
        for c, r, ln in _win_slices(r0, r1):
            nc.sync.dma_start(
                out=win[:, c : c + ln], in_=fp[:, r + H2 : r + H2 + ln]
            )

    with tile.TileContext(nc) as tc:
        nc.gpsimd.load_library(library_config.ap_gather)
        with (
            tc.tile_pool(name="const", bufs=1) as const_pool,
            tc.tile_pool(name="idxp", bufs=2) as idx_pool,
            tc.tile_pool(name="h", bufs=3) as h_pool,
            tc.tile_pool(name="o", bufs=2) as o_pool,
            tc.tile_pool(name="ps", bufs=4, space="PSUM") as psum_pool,
        ):
            w_sb = const_pool.tile([P, NBLK * OUTC], F32)
            nc.sync.dma_start(out=w_sb[:], in_=wstk[:])
            if bf16_mm:
                wb16 = const_pool.tile([P, NBLK * OUTC], mybir.dt.bfloat16)
                nc.scalar.copy(out=wb16[:], in_=w_sb[:])
            else:
                wb16 = None

            wins = [const_pool.tile([P, WINQ], F32, name=f"win{i}") for i in range(2)]
            for i, w in enumerate(wins):
                nc.scalar.memzero(w[:, 0:1])

            for rep in range(treps):
                for i, w in enumerate(wins):
                    # full-window fill in virtual-row space; win i first serves s=i
                    upd_window(w, _cover_hi(i) - CIRC, _cover_hi(i))
                _body(nc, tc, fp, idx, outT, w_sb, wins,
                      idx_pool, h_pool, o_pool, psum_pool, upd_window,
                      no_mm=no_mm, win_elems=win_elems, wb16=wb16)
    nc.compile()
    return nc


def _body(nc, tc, fp, idx, outT, w_sb, wins,
          idx_pool, h_pool, o_pool, psum_pool, upd_window,
          no_mm=False, win_elems=WINQ, wb16=None):
    if True:  # keep indentation shallow
        if True:
            for s in range(NSUP):
                win = wins[s % 2]
                if s >= 2:
                    # this window last served supertile s-2; roll it forward
                    upd_window(win, _cover_hi(s - 2), _cover_hi(s))

                it = idx_pool.tile([P, NIDX // 16], I16, tag="it")
                nc.scalar.dma_start(out=it[:], in_=idx[s])

                H = h_pool.tile([P, NIDX], F32, tag="H")
                nc.gpsimd.ap_gather(
                    out_ap=H[:].rearrange("p (n d) -> p n d", d=1),
                    in_ap=win[:].rearrange("p (n d) -> p n d", d=1)[:, :win_elems],
                    idxs_ap=it[:],
                    channels=P,
                    num_elems=win_elems,
                    d=1,
                    num_idxs=NIDX,
                )
                if no_mm:
                    continue
                if wb16 is not None:
                    Hb = h_pool.tile([P, NIDX], mybir.dt.bfloat16, tag="Hb")
                    # cast on the otherwise-idle DVE so ACT only does relu
                    nc.vector.tensor_scalar_add(Hb[:], H[:], 0.0)
                    Hm, Wm = Hb, wb16
                else:
                    Hm, Wm = H, w_sb

                ps = psum_pool.tile([OUTC, SUPER], F32, tag="ps")
                for h in range(2):  # matmul output must fit one PSUM bank
                    for b in range(NBLK):
                        nc.tensor.matmul(
                            ps[:, 512 * h : 512 * h + 512],
                            lhsT=Wm[:, b * OUTC : (b + 1) * OUTC],
                            rhs=Hm[:, b * SUPER + 512 * h : b * SUPER + 512 * h + 512],
                            start=(b == 0),
                            stop=(b == NBLK - 1),
                        )

                o_sb = o_pool.tile([OUTC, SUPER], F32, tag="o")
                nc.scalar.activation(
                    out=o_sb[:], in_=ps[:],
                    func=mybir.ActivationFunctionType.Relu,
                )
                nc.sync.dma_start(
                    out=outT[:, s * SUPER : (s + 1) * SUPER], in_=o_sb[:]
                )


# --- host prep ------------------------------------------------------------


def recon_order(kmap):
    """Raster voxel order reconstructed from kmap via BFS coord propagation."""
    from scipy import sparse
    from scipy.sparse import csgraph

    km = np.asarray(kmap)
    n = km.shape[1]
    offs = np.array(
        [[dx, dy, dz] for dx in (-1, 0, 1) for dy in (-1, 0, 1) for dz in (-1, 0, 1)],
        dtype=np.int32,
    )
    src = np.repeat(np.arange(n, dtype=np.int32)[None, :], K3, axis=0).ravel()
    dst = km.ravel()
    kk = np.repeat(np.arange(K3, dtype=np.int32)[:, None], n, axis=1).ravel()
    m = (dst < n) & (kk != 13)
    src, dst, kk = src[m], dst[m], kk[m]

    G = sparse.csr_matrix((np.ones(src.size, np.int8), (src, dst)), shape=(n, n))
    ncomp, labels = csgraph.connected_components(G, directed=False)

    eorder = np.argsort(src, kind="stable")
    esrc, edst, ek = src[eorder], dst[eorder], kk[eorder]
    eptr = np.searchsorted(esrc, np.arange(n + 1)).astype(np.int64)
    doff = offs[ek]

    order_scan = np.argsort(labels, kind="stable")
    starts = np.searchsorted(labels[order_scan], np.arange(ncomp))
    roots = order_scan[starts]

    coord = np.zeros((n, 3), dtype=np.int32)
    visited = np.zeros(n, dtype=bool)
    visited[roots] = True
    frontier = roots
    while frontier.size:
        cnt = eptr[frontier + 1] - eptr[frontier]
        tot = int(cnt.sum())
        if tot == 0:
            break
        base = np.repeat(eptr[frontier], cnt)
        idx = base + (np.arange(tot) - np.repeat(np.cumsum(cnt) - cnt, cnt))
        ds = edst[idx]
        ncrd = coord[np.repeat(frontier, cnt)] + doff[idx]
        fresh = ~visited[ds]
        ds_f, nc_f = ds[fresh], ncrd[fresh]
        uniq, ui = np.unique(ds_f, return_index=True)
        coord[uniq] = nc_f[ui]
        visited[uniq] = True
        frontier = uniq
    assert visited.all(), "kmap graph BFS did not reach all voxels"

    cmin = np.zeros((ncomp, 3), np.int32)
    np.minimum.at(cmin, labels, coord)
    coord -= cmin[labels]
    ext = coord.max(0).astype(np.int64) + 1
    lin_r = (coord[:, 0].astype(np.int64) * ext[1] + coord[:, 1]) * ext[2] + coord[:, 2]
    return np.lexsort((lin_r, labels))


def host_prep(feats, weight, kmap, order):
    n = feats.shape[0]
    feats = np.asarray(feats, dtype=np.float32)
    km = np.asarray(kmap, dtype=np.int32)

    rank = np.empty(n, dtype=np.int64)
    rank[order] = np.arange(n)
    feats_sorted = np.zeros((NTOT, INC), dtype=np.float32)
    feats_sorted[:n] = feats[order]

    # gpos[k, q]: sorted row of the k-tap of the voxel at sorted position q
    km_sorted = np.full((K3, NTOT), n, dtype=np.int64)
    km_sorted[:, :n] = km[:, order]
    gpos = np.where(km_sorted < n, rank[np.minimum(km_sorted, n - 1)], -1)

    deltas = gpos - np.arange(NTOT)[None, :]
    band = int(np.abs(deltas[gpos >= 0]).max())
    assert band < BAND, f"rank band {band} exceeds BAND {BAND}"
    # per-quarter delta ranges -> per-quarter virtual-row shift HIM_q
    him = np.zeros(4, dtype=np.int64)
    for q in range(4):
        dq = deltas[QTAPS[q]][gpos[QTAPS[q]] >= 0]
        him[q] = int(dq.max()) + 64
        span = 1024 + int(dq.max()) - int(dq.min()) + 128
        assert span + 2048 + 128 <= CIRC, f"quarter {q} span {span} too wide"
    him_k = np.zeros(K3, dtype=np.int64)
    for q in range(4):
        for k in QTAPS[q]:
            him_k[k] = him[q]

    # stacked weights: block b rows 32q..32q+31 = W[QTAPS[q][b]]
    w = np.asarray(weight, dtype=np.float32)
    wstk = np.zeros((P, NBLK * OUTC), dtype=np.float32)
    for q in range(4):
        for b, k in enumerate(QTAPS[q]):
            wstk[32 * q : 32 * q + 32, b * OUTC : (b + 1) * OUTC] = w[k]

    in_maps = []
    for c in range(NCORES):
        lo = c * NPC
        # FT padded: core-local rows [-HFT, NPC+HFT)
        g0, g1 = lo - HFT, lo + NPC + HFT
        ftp = np.zeros((NPC + 2 * HFT, INC), dtype=np.float32)
        a, b_ = max(0, g0), min(NTOT, g1)
        ftp[a - g0 : b_ - g0] = feats_sorted[a:b_]
        # slab in virtual-row space, per-quarter shift: slab col j of quarter
        # q holds real row (j - H2) - BAND + him[q]
        fp_c = np.empty((P, SLAB), dtype=np.float32)
        j = np.arange(SLAB)
        for q in range(4):
            r = j - H2 - BAND + him[q]
            fp_c[32 * q : 32 * q + 32, :] = ftp[r + HFT].T
        fp_c = np.ascontiguousarray(fp_c)

        # gather indices: real row -> virtual row -> circular window col
        gp = gpos[:, lo : lo + NPC]  # [27, NPC] absolute rows
        rloc = gp - lo
        valid = gp >= 0
        v = rloc + BAND - him_k[:, None]
        s_of = np.arange(NPC) // SUPER
        vlo = (s_of + 1) * SUPER + BAND - CIRC   # exclusive lower bound
        ok = (~valid) | ((v > vlo[None, :]) & (v <= (s_of[None, :] + 1) * SUPER + BAND))
        assert ok.all(), "virtual row outside live window"
        wcol = np.where(valid, 1 + (v % CIRC), 0).astype(np.int16)

        idx_c = np.zeros((NSUP, P, NIDX // 16), dtype=np.int16)
        j = np.arange(NIDX)
        for s in range(NSUP):
            for q in range(4):
                stream = np.zeros((NBLK, SUPER), dtype=np.int16)
                for b, k in enumerate(QTAPS[q]):
                    stream[b] = wcol[k, s * SUPER : (s + 1) * SUPER]
                flat = stream.reshape(-1)
                wrap = np.zeros((16, NIDX // 16), dtype=np.int16)
                wrap[j % 16, j // 16] = flat
                idx_c[s, 32 * q : 32 * q + 16] = wrap
                idx_c[s, 32 * q + 16 : 32 * q + 32] = wrap

        in_maps.append(
            {"fp": fp_c, "idx": idx_c, "wstk": wstk,
             "outT": np.zeros((OUTC, NPC), np.float32)}
        )
    return in_maps


def unshard(results, n, order):
    outs = [r["outT"].T for r in results]
    out_sorted = np.concatenate(outs, axis=0)
    out = np.empty((n, OUTC), dtype=np.float32)
    out[order] = out_sorted[:n]
    return out


_LAST_NC = None


def run(feats, weight, kmap, **kw):
    n = feats.shape[0]
    order = recon_order(kmap)
    in_maps = host_prep(feats, weight, kmap, order)
    nc = build_nc()
    res = run_bass_kernel_spmd(nc, in_maps, core_ids=list(range(NCORES)), **kw)
    out = unshard(res.results, n, order)
    return out, res


def kernel(feats, weight, kmap):
    out, _ = run(feats, weight, kmap)
    return out


# revision 4
# speedup vs baseline: 1.0096x; 1.0096x over previous
"""Sparse 3D conv (gather -> matmul -> relu) via GPSIMD ap_gather, 8 cores.

out[n] = relu(sum_k feats[kmap[k,n]] @ W[k]), sentinel index N contributes 0.

Plan (data-parallel over voxels, no collectives):
  HOST:
    - Reconstruct a raster (z-order) voxel ordering from kmap alone: BFS over
      the 26-neighbor graph propagates exact (x,y,z) offsets, so each
      connected component gets consistent coords; sort by (component, lin).
      Neighbor rank deltas are then bounded by ~3300 (measured 3292), and the
      27 taps cluster by dx: deltas in [-3292,-1], [-46,46], [1,3292].
    - Taps are split into 4 quarters of 7 blocks (Q3: 6 + pad). Each quarter
      sees only its own delta band, so its window spans ~4500 rows. A shared
      "virtual row" v = real_row + BAND - HIM_q absorbs the per-quarter shift
      HIM_q (= max delta + 64) into the host-built slab fp[128, SLAB]: rows
      32q+c hold channel c shifted for quarter q. The device schedule is
      therefore identical on all cores and quarters (SPMD-uniform).
    - Per supertile of 1024 voxels: int16 circular-window gather indices
      1 + (v % CIRC); invalid taps -> 0 (a permanently-zero window column).
  DEVICE (per supertile s):
    - Rolling circular window W[128, 1+CIRC=6913] f32, A/B alternating so
      DMA rolls (2048 virtual rows, one wide dma_start) overlap gathers.
    - ONE gpsimd.ap_gather (channels=128, d=1, num_idxs=7168): each 16-lane
      Q7 core gathers its quarter's tap stream; H[128, 7*1024] lands
      matmul-ready. The window (6913) is smaller than the output (7168), so
      the Pool-engine charge is output-driven -- the floor for this op.
    - DVE casts H f32 -> bf16 (keeps ACT free of head-of-line blocking
      behind the PE); 14 matmuls K=128 x 512 cols (bf16, one PSUM bank
      each): stationary = 4 stacked tap weights [128, 64], PSUM accumulates
      all 7 blocks; ACT relu PSUM -> SBUF; DMA out as outT[64, positions].
  HOST: transpose, un-permute, drop pad rows.
"""

import numpy as np

import concourse.bass as bass
import concourse.mybir as mybir
import concourse.tile as tile
from concourse import bacc, library_config
from concourse.bass_utils import run_bass_kernel_spmd

# --- tail-drain wait splitting (same workaround as baseline kernel) --------


def _split_drain_and_barrier(self, tick_clock, wait_clock):
    nc = self.nc
    collector = nc.sync.nop(nofuse=True)
    wait_clock.add_sem_waits(
        collector.ins, tile.ScopedClock({None: tick_clock.global_clock})
    )
    si = collector.ins.sync_info
    waits = list(si.on_wait) if si is not None and si.on_wait else []
    if len(waits) > 1:
        collector.ins.sync_info = mybir.SyncInfo(
            on_wait=waits[:1], on_update=list(si.on_update or [])
        )
        for w in waits[1:]:
            extra = nc.sync.nop(nofuse=True)
            extra.ins.sync_info = mybir.SyncInfo(on_wait=[w], on_update=[])
    nc.sync.drain()
    nc.all_engine_barrier()
    popped = nc._tile_sem_poison_stack.pop()
    assert popped is self._sem_poison
    nc.clear_and_free_semaphores(list(self.sems.allocated().values()))
    nc.all_engine_barrier()


tile.TileContext._drain_and_barrier = _split_drain_and_barrier

# --- problem constants ----------------------------------------------------
N = 400000
INC = 32
OUTC = 64
K3 = 27
NCORES = 8
P = 128

SUPER = 1024
NSUP = 49
NPC = NSUP * SUPER          # 50176 voxels per core
NTOT = NCORES * NPC         # 401408 padded voxel count

NBLK = 7                    # tap blocks per quarter (Q3: 6 real + 1 pad)
NIDX = NBLK * SUPER         # 7168 gather indices per Q7 core per supertile
QTAPS = [list(range(0, 7)), list(range(7, 14)),
         list(range(14, 21)), list(range(21, 27))]

BAND = 3456                 # virtual-row reach above a supertile
CIRC = 6912                 # circular window length (cols 1..CIRC)
WINQ = CIRC + 1             # + permanently-zero col 0
H2 = 2560                   # slab col offset: slab col = virtual row + H2
SLAB = NSUP * SUPER + BAND + H2 + 64   # virtual-row slab columns
HFT = 6144                  # FT halo for per-quarter shifted slab builds

F32 = mybir.dt.float32
F32R = mybir.dt.float32r
I16 = mybir.dt.int16


def _cover_hi(s):
    return (s + 1) * SUPER + BAND  # highest row covered after update s


def _win_slices(r0, r1):
    """Rows [r0, r1) -> list of (win_col_start, slab_rel_start, length)."""
    out = []
    r = r0
    while r < r1:
        c = 1 + (r % CIRC)
        ln = min(r1 - r, CIRC + 1 - c)
        out.append((c, r, ln))
        r += ln
    return out


def build_nc(treps=1, no_mm=False, win_elems=WINQ, bf16_mm=True):
    nc = bacc.Bacc("TRN2", target_bir_lowering=False, debug=False)
    fp = nc.declare_dram_parameter("fp", [P, SLAB], F32, isOutput=False)
    idx = nc.declare_dram_parameter("idx", [NSUP, P, NIDX // 16], I16, isOutput=False)
    wstk = nc.declare_dram_parameter("wstk", [P, NBLK * OUTC], F32, isOutput=False)
    outT = nc.declare_dram_parameter("outT", [OUTC, NPC], F32, isOutput=True)

    def upd_window(win, r0, r1):
        """DMA slab rows [r0, r1) into circular window cols (slab is already
        replicated x4 across the 128 partitions by the host)."""
        for c, r, ln in _win_slices(r0, r1):
            nc.sync.dma_start(
                out=win[:, c : c + ln], in_=fp[:, r + H2 : r + H2 + ln]
            )

    with tile.TileContext(nc) as tc:
        nc.gpsimd.load_library(library_config.ap_gather)
        with (
            tc.tile_pool(name="const", bufs=1) as const_pool,
            tc.tile_pool(name="idxp", bufs=2) as idx_pool,
            tc.tile_pool(name="h", bufs=3) as h_pool,
            tc.tile_pool(name="o", bufs=2) as o_pool,
            tc.tile_pool(name="ps", bufs=4, space="PSUM") as psum_pool,
        ):
            w_sb = const_pool.tile([P, NBLK * OUTC], F32)
            nc.sync.dma_start(out=w_sb[:], in_=wstk[:])
            if bf16_mm:
                wb16 = const_pool.tile([P, NBLK * OUTC], mybir.dt.bfloat16)
                nc.scalar.copy(out=wb16[:], in_=w_sb[:])
            else:
                wb16 = None

            wins = [const_pool.tile([P, WINQ], F32, name=f"win{i}") for i in range(2)]
            for i, w in enumerate(wins):
                nc.scalar.memzero(w[:, 0:1])

            for rep in range(treps):
                for i, w in enumerate(wins):
                    # full-window fill in virtual-row space; win i first serves s=i
                    upd_window(w, _cover_hi(i) - CIRC, _cover_hi(i))
                _body(nc, tc, fp, idx, outT, w_sb, wins,
                      idx_pool, h_pool, o_pool, psum_pool, upd_window,
                      no_mm=no_mm, win_elems=win_elems, wb16=wb16)
    nc.compile()
    return nc


def _body(nc, tc, fp, idx, outT, w_sb, wins,
          idx_pool, h_pool, o_pool, psum_pool, upd_window,
          no_mm=False, win_elems=WINQ, wb16=None):
    if True:  # keep indentation shallow
        if True:
            for s in range(NSUP):
                win = wins[s % 2]
                if s >= 2:
                    # this window last served supertile s-2; roll it forward
                    upd_window(win, _cover_hi(s - 2), _cover_hi(s))

                it = idx_pool.tile([P, NIDX // 16], I16, tag="it")
                nc.scalar.dma_start(out=it[:], in_=idx[s])

                H = h_pool.tile([P, NIDX], F32, tag="H")
                nc.gpsimd.ap_gather(
                    out_ap=H[:].rearrange("p (n d) -> p n d", d=1),
                    in_ap=win[:].rearrange("p (n d) -> p n d", d=1)[:, :win_elems],
                    idxs_ap=it[:],
                    channels=P,
                    num_elems=win_elems,
                    d=1,
                    num_idxs=NIDX,
                )
                if no_mm:
                    continue
                if wb16 is not None:
                    Hb = h_pool.tile([P, NIDX], mybir.dt.bfloat16, tag="Hb")
                    # cast on the otherwise-idle DVE so ACT only does relu
                    nc.vector.tensor_scalar_add(Hb[:], H[:], 0.0)
                    Hm, Wm = Hb, wb16
                else:
                    Hm, Wm = H, w_sb

                ps = psum_pool.tile([OUTC, SUPER], F32, tag="ps")
                for h in range(2):  # matmul output must fit one PSUM bank
                    for b in range(NBLK):
                        nc.tensor.matmul(
                            ps[:, 512 * h : 512 * h + 512],
                            lhsT=Wm[:, b * OUTC : (b + 1) * OUTC],
                            rhs=Hm[:, b * SUPER + 512 * h : b * SUPER + 512 * h + 512],
                            start=(b == 0),
                            stop=(b == NBLK - 1),
                        )

                o_sb = o_pool.tile([OUTC, SUPER], F32, tag="o")
                nc.scalar.activation(
                    out=o_sb[:], in_=ps[:],
                    func=mybir.ActivationFunctionType.Relu,
                )
                nc.sync.dma_start(
                    out=outT[:, s * SUPER : (s + 1) * SUPER], in_=o_sb[:]
                )


# --- host prep ------------------------------------------------------------


def recon_order(kmap):
    """Raster voxel order reconstructed from kmap via BFS coord propagation."""
    from scipy import sparse
    from scipy.sparse import csgraph

    km = np.asarray(kmap)
    n = km.shape[1]
    offs = np.array(
        [[dx, dy, dz] for dx in (-1, 0, 1) for dy in (-1, 0, 1) for dz in (-1, 0, 1)],
        dtype=np.int32,
    )
    src = np.repeat(np.arange(n, dtype=np.int32)[None, :], K3, axis=0).ravel()
    dst = km.ravel()
    kk = np.repeat(np.arange(K3, dtype=np.int32)[:, None], n, axis=1).ravel()
    m = (dst < n) & (kk != 13)
    src, dst, kk = src[m], dst[m], kk[m]

    G = sparse.csr_matrix((np.ones(src.size, np.int8), (src, dst)), shape=(n, n))
    ncomp, labels = csgraph.connected_components(G, directed=False)

    eorder = np.argsort(src, kind="stable")
    esrc, edst, ek = src[eorder], dst[eorder], kk[eorder]
    eptr = np.searchsorted(esrc, np.arange(n + 1)).astype(np.int64)
    doff = offs[ek]

    order_scan = np.argsort(labels, kind="stable")
    starts = np.searchsorted(labels[order_scan], np.arange(ncomp))
    roots = order_scan[starts]

    coord = np.zeros((n, 3), dtype=np.int32)
    visited = np.zeros(n, dtype=bool)
    visited[roots] = True
    frontier = roots
    while frontier.size:
        cnt = eptr[frontier + 1] - eptr[frontier]
        tot = int(cnt.sum())
        if tot == 0:
            break
        base = np.repeat(eptr[frontier], cnt)
        idx = base + (np.arange(tot) - np.repeat(np.cumsum(cnt) - cnt, cnt))
        ds = edst[idx]
        ncrd = coord[np.repeat(frontier, cnt)] + doff[idx]
        fresh = ~visited[ds]
        ds_f, nc_f = ds[fresh], ncrd[fresh]
        uniq, ui = np.unique(ds_f, return_index=True)
        coord[uniq] = nc_f[ui]
        visited[uniq] = True
        frontier = uniq
    assert visited.all(), "kmap graph BFS did not reach all voxels"

    cmin = np.zeros((ncomp, 3), np.int32)
    np.minimum.at(cmin, labels, coord)
    coord -= cmin[labels]
    ext = coord.max(0).astype(np.int64) + 1
    lin_r = (coord[:, 0].astype(np.int64) * ext[1] + coord[:, 1]) * ext[2] + coord[:, 2]
    return np.lexsort((lin_r, labels))


def host_prep(feats, weight, kmap, order):
    n = feats.shape[0]
    feats = np.asarray(feats, dtype=np.float32)
    km = np.asarray(kmap, dtype=np.int32)

    rank = np.empty(n, dtype=np.int64)
    rank[order] = np.arange(n)
    feats_sorted = np.zeros((NTOT, INC), dtype=np.float32)
    feats_sorted[:n] = feats[order]

    # gpos[k, q]: sorted row of the k-tap of the voxel at sorted position q
    km_sorted = np.full((K3, NTOT), n, dtype=np.int64)
    km_sorted[:, :n] = km[:, order]
    gpos = np.where(km_sorted < n, rank[np.minimum(km_sorted, n - 1)], -1)

    deltas = gpos - np.arange(NTOT)[None, :]
    band = int(np.abs(deltas[gpos >= 0]).max())
    assert band < BAND, f"rank band {band} exceeds BAND {BAND}"
    # per-quarter delta ranges -> per-quarter virtual-row shift HIM_q
    him = np.zeros(4, dtype=np.int64)
    for q in range(4):
        dq = deltas[QTAPS[q]][gpos[QTAPS[q]] >= 0]
        him[q] = int(dq.max()) + 64
        span = 1024 + int(dq.max()) - int(dq.min()) + 128
        assert span + 2048 + 128 <= CIRC, f"quarter {q} span {span} too wide"
    him_k = np.zeros(K3, dtype=np.int64)
    for q in range(4):
        for k in QTAPS[q]:
            him_k[k] = him[q]

    # stacked weights: block b rows 32q..32q+31 = W[QTAPS[q][b]]
    w = np.asarray(weight, dtype=np.float32)
    wstk = np.zeros((P, NBLK * OUTC), dtype=np.float32)
    for q in range(4):
        for b, k in enumerate(QTAPS[q]):
            wstk[32 * q : 32 * q + 32, b * OUTC : (b + 1) * OUTC] = w[k]

    in_maps = []
    for c in range(NCORES):
        lo = c * NPC
        # FT padded: core-local rows [-HFT, NPC+HFT)
        g0, g1 = lo - HFT, lo + NPC + HFT
        ftp = np.zeros((NPC + 2 * HFT, INC), dtype=np.float32)
        a, b_ = max(0, g0), min(NTOT, g1)
        ftp[a - g0 : b_ - g0] = feats_sorted[a:b_]
        # slab in virtual-row space, per-quarter shift: slab col j of quarter
        # q holds real row (j - H2) - BAND + him[q]
        fp_c = np.empty((P, SLAB), dtype=np.float32)
        j = np.arange(SLAB)
        for q in range(4):
            r = j - H2 - BAND + him[q]
            fp_c[32 * q : 32 * q + 32, :] = ftp[r + HFT].T
        fp_c = np.ascontiguousarray(fp_c)

        # gather indices: real row -> virtual row -> circular window col
        gp = gpos[:, lo : lo + NPC]  # [27, NPC] absolute rows
        rloc = gp - lo
        valid = gp >= 0
        v = rloc + BAND - him_k[:, None]
        s_of = np.arange(NPC) // SUPER
        vlo = (s_of + 1) * SUPER + BAND - CIRC   # exclusive lower bound
        ok = (~valid) | ((v > vlo[None, :]) & (v <= (s_of[None, :] + 1) * SUPER + BAND))
        assert ok.all(), "virtual row outside live window"
        wcol = np.where(valid, 1 + (v % CIRC), 0).astype(np.int16)

        idx_c = np.zeros((NSUP, P, NIDX // 16), dtype=np.int16)
        j = np.arange(NIDX)
        for s in range(NSUP):
            for q in range(4):
                stream = np.zeros((NBLK, SUPER), dtype=np.int16)
                for b, k in enumerate(QTAPS[q]):
                    stream[b] = wcol[k, s * SUPER : (s + 1) * SUPER]
                flat = stream.reshape(-1)
                wrap = np.zeros((16, NIDX // 16), dtype=np.int16)
                wrap[j % 16, j // 16] = flat
                idx_c[s, 32 * q : 32 * q + 16] = wrap
                idx_c[s, 32 * q + 16 : 32 * q + 32] = wrap

        in_maps.append(
            {"fp": fp_c, "idx": idx_c, "wstk": wstk,
             "outT": np.zeros((OUTC, NPC), np.float32)}
        )
    return in_maps


def unshard(results, n, order):
    outs = [r["outT"].T for r in results]
    out_sorted = np.concatenate(outs, axis=0)
    out = np.empty((n, OUTC), dtype=np.float32)
    out[order] = out_sorted[:n]
    return out


_LAST_NC = None


def run(feats, weight, kmap, **kw):
    n = feats.shape[0]
    order = recon_order(kmap)
    in_maps = host_prep(feats, weight, kmap, order)
    nc = build_nc()
    res = run_bass_kernel_spmd(nc, in_maps, core_ids=list(range(NCORES)), **kw)
    out = unshard(res.results, n, order)
    return out, res


def kernel(feats, weight, kmap):
    out, _ = run(feats, weight, kmap)
    return out
